# revision 4
# baseline (speedup 1.0000x reference)
"""Trainium2 Bass kernel for nn_MoCA (self-attention + momentum concept
attention), v2.

Sharding: pure data parallel - batch dim (B=8) sharded 1 batch per core,
weights/concepts replicated. No collectives.

Per-core algorithm for one batch (C=512, L=64, HW=4096, P=256), restructured
algebraically vs the reference:
  out = wosa^T @ lat + womo^T @ lat2 + fm          (single fused residual)
  S2  = (m2 @ fm) + (wosa @ m2^T) @ lat            (MoCA scores; m2 @ fm is
        precomputed in the head phase, so MoCA needs no sa_out tensor)
Engine plan:
  - th/ph stored bf16 duplicated across partition halves; S^T computed as
    2 concurrent 64-contraction row-tiles (tile_position) per 128-m-chunk.
  - exp of S^T alternates between ACT (AFT.Exp) and DVE (Schraudolph
    2^x bit-trick: i = round(s*184.664 + B) as uint16, bitcast to bf16),
    halving the softmax elementwise wall.
  - PV accumulates [g^T | 1] @ P^T so denominators ride along; normalize
    after a PE transpose; the torch .view reshape is realized by a DRAM
    round trip (sc1/sc2) with batched DMAs.
"""
import sys

if '/opt/trn_rl_repo' not in sys.path:
    sys.path.insert(0, '/opt/trn_rl_repo')

import numpy as np

C, L, H, W, P = 512, 64, 64, 64, 256
HW = H * W
B = 8
N_CORES = 8

NB = HW // 512          # 8 n-blocks of 512
NM = HW // 128          # 32 m-chunks of 128
NCC = C // 128          # 4 channel chunks

# Schraudolph exp-as-bits constants (bf16 target):
#   bits = round(y * log2(e) * 128 + (16256 - 7.4))
EXP_A = float(np.log2(np.e) * 128.0)
EXP_B = 16256.0 - 7.4

_STATE: dict = {}


def _build_program(reps=1, num_devices=N_CORES):
    import concourse.bass as bass
    import concourse.bacc as bacc
    import concourse.mybir as mybir
    from concourse import tile
    from concourse.masks import make_identity

    dt = mybir.dt
    AFT = mybir.ActivationFunctionType
    ALU = mybir.AluOpType
    f32, f32r, bf16 = dt.float32, dt.float32r, dt.bfloat16
    u16 = dt.uint16

    nc = bacc.Bacc("TRN2", target_bir_lowering=False, debug=False,
                   enable_asserts=False, num_devices=num_devices)

    fm_d = nc.dram_tensor("fm", [C, HW], f32r, kind="ExternalInput").ap()
    # [wth 64 | wph 64 | wg 64]
    wb_d = nc.dram_tensor("wb", [C, 192], f32r, kind="ExternalInput").ap()
    # [wosa C | womo C | a2 P | conceptsT P]  (64 rows)
    ws_d = nc.dram_tensor("ws", [L, 2 * C + 2 * P], bf16,
                          kind="ExternalInput").ap()
    p2w_d = nc.dram_tensor("p2w", [128, 130], bf16, kind="ExternalInput").ap()
    out_d = nc.dram_tensor("out", [C, HW], f32, kind="ExternalOutput").ap()

    with tile.TileContext(nc) as tc:
      for _rep in range(reps):
        with tc.tile_pool(name="sb", bufs=1) as sb, \
             tc.tile_pool(name="dram", bufs=1, space="DRAM") as dp, \
             tc.tile_pool(name="ps", bufs=1, space="PSUM") as psum:

            sc1 = dp.tile([HW, L], bf16, tag="sc1", name="sc1")
            sc2 = dp.tile([HW, L], bf16, tag="sc2", name="sc2")

            # ---------------- persistent tiles ----------------
            fmr = [sb.tile([128, HW], f32r, tag=f"fmr{i}", name=f"fmr{i}")
                   for i in range(NCC)]
            wb = [sb.tile([128, 192], f32r, tag=f"wb{i}", name=f"wb{i}")
                  for i in range(NCC)]
            wsb = sb.tile([L, 2 * C + 2 * P], bf16, tag="wsb", name="wsb")
            p2w = sb.tile([128, 130], bf16, tag="p2w", name="p2w")
            thd = sb.tile([128, HW], bf16, tag="thd", name="thd")
            phd = sb.tile([128, HW], bf16, tag="phd", name="phd")
            gsb = sb.tile([L, HW], bf16, tag="gsb", name="gsb")
            gto = sb.tile([128, NM * 65], bf16, tag="gto", name="gto")
            lat = sb.tile([L, HW], bf16, tag="lat", name="lat")
            lat2 = sb.tile([L, HW], bf16, tag="lat2", name="lat2")
            id64b = sb.tile([64, 64], bf16, tag="id64b", name="id64b")
            id65 = sb.tile([65, 65], f32, tag="id65", name="id65")

            make_identity(nc, id64b[:])
            make_identity(nc, id65[:])
            nc.vector.memset(gto[:], 1.0)

            # ---------------- phase 1: loads ----------------
            # weights first, then fm in (nb, ci) chunks so the first conv
            # accumulation chain can start ~5us in rather than after the
            # whole 8MB of fm has landed.
            for ci in range(NCC):
                nc.sync.dma_start(wb[ci][:],
                                  wb_d[ci * 128:(ci + 1) * 128, :])
            for nb in range(NB):
                ns = slice(nb * 512, (nb + 1) * 512)
                for ci in range(NCC):
                    nc.sync.dma_start(fmr[ci][:, ns],
                                      fm_d[ci * 128:(ci + 1) * 128, ns])
            nc.sync.dma_start(wsb[:], ws_d[:])
            nc.sync.dma_start(p2w[:], p2w_d[:])

            # -------- phases 2+3: head convs interleaved with SA --------
            # The head phase is fm-DMA-bound, so SA pair-steps for nb=0 are
            # interleaved as soon as their phd columns have landed — the PE
            # never idles waiting on the fm stream.
            # SA is a flat software pipeline over (nb, j) pair-steps: PV
            # lags the (S^T, exp) producer by LAG steps and flows across nb
            # boundaries so the in-order PE queue never drains.
            NPAIR = NM // 2
            LAG = 2
            sc1_v = sc1[:].rearrange("(a k p) c -> a p k c", a=NB, k=4)
            lat_view = sc1[:].rearrange("(a b) c -> a (b c)", a=L)
            steps = [(nb, j) for nb in range(NB) for j in range(NPAIR)]
            pvs = {}
            pts = {}
            sa_pos = [0]

            def _sa_normalize(nb):
                pv = pvs.pop(nb)
                at = sb.tile([65, 512], f32, tag="at", name="at", bufs=2)
                nc.scalar.activation(at[:], pv[:], AFT.Copy)
                stg = sb.tile([128, 256], bf16, tag="stg", name="stg", bufs=2)
                for k in range(4):
                    tp = psum.tile([128, 65], f32, tag="sm", name="tt",
                                   bufs=2, padded_shape=[128, 512])
                    nc.tensor.transpose(tp[:], at[:, k * 128:(k + 1) * 128],
                                        id65[:])
                    rc = sb.tile([128, 1], f32, tag="rc", name="rc", bufs=4)
                    nc.vector.reciprocal(rc[:], tp[:, 64:65])
                    nc.scalar.activation(stg[:, k * 64:(k + 1) * 64],
                                         tp[:, 0:64], AFT.Copy, scale=rc[:])
                nc.sync.dma_start(
                    sc1_v[nb],
                    stg[:].rearrange("p (k c) -> p k c", k=4))
                if nb % 2 == 1:
                    q = nb // 2
                    nc.sync.dma_start(lat[16 * q:16 * (q + 1), :],
                                      lat_view[16 * q:16 * (q + 1), :])

            def _sa_advance(limit):
                while sa_pos[0] < min(limit, len(steps) + LAG):
                    idx = sa_pos[0]
                    sa_pos[0] += 1
                    if idx < len(steps):
                        nb, j = steps[idx]
                        ns = slice(nb * 512, (nb + 1) * 512)
                        st = psum.tile([128, 1024], f32, tag="big", name="st",
                                       bufs=3)
                        for h in range(2):
                            mc = 2 * j + h
                            hp = slice(64 * h, 64 * h + 64)
                            nc.tensor.matmul(
                                st[:, h * 512:(h + 1) * 512],
                                phd[hp, mc * 128:(mc + 1) * 128],
                                thd[hp, ns], start=True, stop=True,
                                tile_position=(64 * h, 0))
                        if j % 2 == 0:
                            ptt = sb.tile([128, 1024], bf16, tag="pt",
                                          name="pt", bufs=LAG + 4)
                            nc.scalar.activation(ptt[:], st[:], AFT.Exp)
                            pts[idx] = ptt[:]
                        else:
                            ptu = sb.tile([128, 1024], u16, tag="ptu",
                                          name="ptu", bufs=LAG + 4)
                            nc.vector.tensor_scalar(
                                ptu[:], st[:], EXP_A, EXP_B,
                                ALU.mult, ALU.add)
                            pts[idx] = ptu[:].bitcast(bf16)
                    if idx >= LAG:
                        nb, jj = steps[idx - LAG]
                        if jj == 0:
                            pvs[nb] = psum.tile([65, 512], f32, tag="sm",
                                                name="pv", bufs=2,
                                                padded_shape=[128, 512])
                        pa = pts.pop(idx - LAG)
                        for h in range(2):
                            mc = 2 * jj + h
                            nc.tensor.matmul(
                                pvs[nb][:], gto[:, mc * 65:(mc + 1) * 65],
                                pa[:, h * 512:(h + 1) * 512],
                                start=(mc == 0), stop=(mc == NM - 1))
                        if jj == NPAIR - 1:
                            _sa_normalize(nb)

            for hk in range(NB):
                ns = slice(hk * 512, (hk + 1) * 512)
                # fused qk conv: psum rows 0:64 = th, 64:128 = ph
                qq = psum.tile([128, 512], f32, tag="big", name="qq", bufs=3,
                               padded_shape=[128, 1024])
                for ci in range(NCC):
                    nc.tensor.matmul(qq[:], wb[ci][:, 0:128],
                                     fmr[ci][:, ns],
                                     start=(ci == 0), stop=(ci == NCC - 1))
                nc.scalar.activation(thd[0:64, ns], qq[0:64, :], AFT.Copy)
                nc.vector.tensor_copy(phd[64:128, ns], qq[64:128, :])
                # replicate across partition halves via SWDGE (Pool queue)
                nc.gpsimd.dma_start(thd[64:128, ns], thd[0:64, ns])
                nc.gpsimd.dma_start(phd[0:64, ns], phd[64:128, ns])
                gp = psum.tile([L, 512], f32, tag="big", name="gp", bufs=3,
                               padded_shape=[128, 1024])
                for ci in range(NCC):
                    nc.tensor.matmul(gp[:], wb[ci][:, 128:192],
                                     fmr[ci][:, ns],
                                     start=(ci == 0), stop=(ci == NCC - 1))
                nc.scalar.activation(gsb[:, ns], gp[:], AFT.Copy)
                # gT blocks for this nb (trailing ones column kept from memset)
                for mc in range(4 * hk, 4 * hk + 4):
                    tp = psum.tile([128, 64], bf16, tag="sm", name="gtp",
                                   bufs=2, padded_shape=[128, 1024])
                    nc.tensor.transpose(tp[:],
                                        gsb[:, mc * 128:(mc + 1) * 128],
                                        id64b[:])
                    nc.vector.tensor_copy(gto[:, mc * 65:mc * 65 + 64], tp[:])
                # SA pair-steps whose phd columns landed one head block ago
                if hk >= 2:
                    _sa_advance(2 * (hk - 1))
            _sa_advance(len(steps) + LAG)

            # ---------------- phase 4: MoCA scores + attention ----------------
            # 1-nb software pipeline: scores+exp run one nb ahead of PV2 so
            # the in-order PE queue never waits on exp2.
            sc2_v = sc2[:].rearrange("(a k p) c -> a p k c", a=NB, k=4)
            lat2_view = sc2[:].rearrange("(a b) c -> a (b c)", a=L)
            moca = {}
            for nb in range(NB + 1):
                if nb < NB:
                    ns = slice(nb * 512, (nb + 1) * 512)
                    s2 = psum.tile([128, 1024], f32, tag="big", name="s2",
                                   bufs=3)
                    # mf part first (needs only thd, flows during the lat
                    # DMA gate), then the a2 @ lat low-rank update.
                    for pc in range(2):
                        nc.tensor.matmul(
                            s2[:, pc * 512:(pc + 1) * 512],
                            wsb[:, 2 * C + P + pc * 128:
                                2 * C + P + (pc + 1) * 128],
                            thd[0:64, ns], start=True, stop=False)
                    for pc in range(2):
                        nc.tensor.matmul(
                            s2[:, pc * 512:(pc + 1) * 512],
                            wsb[:, 2 * C + pc * 128:2 * C + (pc + 1) * 128],
                            lat[:, ns], start=False, stop=True)
                    if nb % 2 == 0:
                        p2t = sb.tile([128, 1024], bf16, tag="p2t",
                                      name="p2t", bufs=3)
                        nc.scalar.activation(p2t[:], s2[:], AFT.Exp)
                        moca[nb] = p2t[:]
                    else:
                        p2u = sb.tile([128, 1024], u16, tag="p2u", name="p2u",
                                      bufs=3)
                        nc.vector.tensor_scalar(p2u[:], s2[:], EXP_A, EXP_B,
                                                ALU.mult, ALU.add)
                        moca[nb] = p2u[:].bitcast(bf16)
                if nb >= 1:
                    nbl = nb - 1
                    p2a = moca.pop(nbl)
                    pv2 = psum.tile([65, 512], f32, tag="big", name="pv2",
                                    bufs=3, padded_shape=[128, 1024])
                    for pc in range(2):
                        nc.tensor.matmul(pv2[:], p2w[:, pc * 65:(pc + 1) * 65],
                                         p2a[:, pc * 512:(pc + 1) * 512],
                                         start=(pc == 0), stop=(pc == 1))
                    at2 = sb.tile([65, 512], f32, tag="at2", name="at2",
                                  bufs=3)
                    nc.scalar.activation(at2[:], pv2[:], AFT.Copy)
                    stg2 = sb.tile([128, 256], bf16, tag="stg2", name="stg2",
                                   bufs=3)
                    for k in range(4):
                        tp = psum.tile([128, 65], f32, tag="sm", name="tt2",
                                       bufs=2, padded_shape=[128, 512])
                        nc.tensor.transpose(tp[:],
                                            at2[:, k * 128:(k + 1) * 128],
                                            id65[:])
                        rc = sb.tile([128, 1], f32, tag="rc2", name="rc2",
                                     bufs=4)
                        nc.vector.reciprocal(rc[:], tp[:, 64:65])
                        if nbl % 2 == 0:
                            nc.vector.tensor_scalar_mul(
                                stg2[:, k * 64:(k + 1) * 64],
                                tp[:, 0:64], rc[:])
                        else:
                            nc.scalar.activation(
                                stg2[:, k * 64:(k + 1) * 64],
                                tp[:, 0:64], AFT.Copy, scale=rc[:])
                    nc.gpsimd.dma_start(
                        sc2_v[nbl],
                        stg2[:].rearrange("p (k c) -> p k c", k=4))
                    if nbl % 2 == 1:
                        q = nbl // 2
                        nc.sync.dma_start(lat2[16 * q:16 * (q + 1), :],
                                          lat2_view[16 * q:16 * (q + 1), :])

            # ---------------- phase 5: fused o-convs + residual ----------------
            out_v = out_d[:].rearrange("(cc p) hw -> p cc hw", cc=NCC)
            for nb in range(NB):
                ns = slice(nb * 512, (nb + 1) * 512)
                ob = sb.tile([128, 2048], f32, tag="ob", name="ob", bufs=3)
                for cc in range(NCC):
                    if cc % 2 == 0:
                        ps = psum.tile([128, 512], f32, tag="big", name="oc",
                                       bufs=3, padded_shape=[128, 1024])
                    else:
                        ps = psum.tile([128, 512], f32, tag="sm", name="oc2",
                                       bufs=2, padded_shape=[128, 512])
                    nc.tensor.matmul(ps[:], wsb[:, cc * 128:(cc + 1) * 128],
                                     lat[:, ns], start=True, stop=False)
                    nc.tensor.matmul(ps[:],
                                     wsb[:, C + cc * 128:C + (cc + 1) * 128],
                                     lat2[:, ns], start=False, stop=True)
                    os_ = slice(cc * 512, (cc + 1) * 512)
                    if cc % 2 == 0:
                        nc.vector.tensor_add(ob[:, os_], ps[:],
                                             fmr[cc][:, ns])
                    else:
                        tmp = sb.tile([128, 512], f32, tag="rtmp",
                                      name="rtmp", bufs=4)
                        nc.scalar.activation(tmp[:], ps[:], AFT.Copy)
                        nc.gpsimd.tensor_add(ob[:, os_], tmp[:],
                                             fmr[cc][:, ns])
                dq = nc.sync if nb % 2 == 0 else nc.gpsimd
                dq.dma_start(
                    out_v[:, :, ns],
                    ob[:].rearrange("p (cc c) -> p cc c", cc=NCC))

    nc.compile()
    return nc


def _get_runner(reps=1):
    """Build the Bass program once and return a cached jitted SPMD callable."""
    key = ("runner", reps)
    if key in _STATE:
        return _STATE[key]

    import jax
    import numpy as np
    from jax.experimental.shard_map import shard_map
    from jax.sharding import Mesh, PartitionSpec
    import concourse.mybir as mybir
    from concourse import bass2jax

    nc = _build_program(reps=reps)
    bass2jax.install_neuronx_cc_hook()

    partition_name = (nc.partition_id_tensor.name
                      if nc.partition_id_tensor else None)
    in_names, out_names, out_avals, zero_shapes = [], [], [], []
    for alloc in nc.m.functions[0].allocations:
        if not isinstance(alloc, mybir.MemoryLocationSet):
            continue
        name = alloc.memorylocations[0].name
        if alloc.kind == "ExternalInput":
            if name != partition_name:
                in_names.append(name)
        elif alloc.kind == "ExternalOutput":
            out_names.append(name)
            shape = tuple(alloc.tensor_shape)
            dtype = mybir.dt.np(alloc.dtype)
            out_avals.append(jax.core.ShapedArray(shape, dtype))
            zero_shapes.append((shape, dtype))
    n_params = len(in_names)
    all_in_names = list(in_names) + list(out_names)
    if partition_name is not None:
        all_in_names.append(partition_name)

    def _body(*args):
        operands = list(args)
        if partition_name is not None:
            operands.append(bass2jax.partition_id_tensor())
        outs = bass2jax._bass_exec_p.bind(
            *operands,
            out_avals=tuple(out_avals),
            in_names=tuple(all_in_names),
            out_names=tuple(out_names),
            lowering_input_output_aliases=(),
            sim_require_finite=True,
            sim_require_nnan=True,
            nc=nc,
        )
        return tuple(outs)

    devices = jax.devices()[:N_CORES]
    mesh = Mesh(np.asarray(devices), ("core",))
    n_outs = len(out_names)
    donate = tuple(range(n_params, n_params + n_outs))
    sharded = jax.jit(
        shard_map(_body, mesh=mesh,
                  in_specs=(PartitionSpec("core"),) * (n_params + n_outs),
                  out_specs=(PartitionSpec("core"),) * n_outs,
                  check_rep=False),
        donate_argnums=donate, keep_unused=True)

    runner = {
        "nc": nc, "sharded": sharded, "in_names": in_names,
        "out_names": out_names, "zero_shapes": zero_shapes,
        "n_params": n_params,
    }
    _STATE[key] = runner
    return runner


def _prep_in_maps(feature_map, concepts, w_theta, w_phi, w_g, w_o,
                  gamma_sa, gamma_moca):
    import ml_dtypes
    bf16 = ml_dtypes.bfloat16

    feature_map = np.asarray(feature_map, dtype=np.float32)
    concepts = np.asarray(concepts, dtype=np.float32)
    w_theta = np.asarray(w_theta, dtype=np.float32)
    w_phi = np.asarray(w_phi, dtype=np.float32)
    w_g = np.asarray(w_g, dtype=np.float32)
    w_o = np.asarray(w_o, dtype=np.float32)
    gamma_sa = np.float32(gamma_sa)
    gamma_moca = np.float32(gamma_moca)

    gain = np.float32(1.0 / np.sqrt(C))
    gain_o = np.float32(1.0 / np.sqrt(L))

    wth_t = w_theta.T * gain                                        # [C, L]
    wph_t = w_phi.T * gain
    wg_t = w_g.T * gain                                             # [C, L]
    m2 = concepts @ (w_theta * gain)                                # [P, C]
    wosa = w_o.T * (gain_o * gamma_sa)                              # [L, C]
    womo = w_o.T * (gain_o * gamma_moca)                            # [L, C]
    a2 = wosa @ m2.T                                                # [L, P]

    wb = np.ascontiguousarray(np.concatenate(
        [wth_t, wph_t, wg_t], axis=1))                              # [C, 192]
    ws = np.ascontiguousarray(np.concatenate(
        [wosa, womo, a2, concepts.T], axis=1)).astype(bf16)         # [L, 1536]
    p2w = np.ones((128, 130), np.float32)
    for pc in range(2):
        p2w[:, pc * 65:pc * 65 + 64] = concepts[pc * 128:(pc + 1) * 128, :]
    p2w = p2w.astype(bf16)
    fm_flat = feature_map.reshape(B, C, HW)

    in_maps = []
    for b in range(N_CORES):
        in_maps.append({
            "fm": np.ascontiguousarray(fm_flat[b]),
            "wb": wb, "ws": ws, "p2w": p2w,
        })
    return in_maps


def _run(in_maps, concat_override=None):
    r = _get_runner()
    concat_in = [
        (concat_override[name] if concat_override and name in concat_override
         else np.concatenate([np.asarray(in_maps[c][name])
                              for c in range(N_CORES)], axis=0))
        for name in r["in_names"]
    ]
    concat_zeros = [np.zeros((N_CORES * s[0], *s[1:]), d)
                    for (s, d) in r["zero_shapes"]]
    out_arrs = r["sharded"](*concat_in, *concat_zeros)
    per_core = []
    for c in range(N_CORES):
        per_core.append({
            name: np.asarray(out_arrs[i]).reshape(
                N_CORES, *r["zero_shapes"][i][0])[c]
            for i, name in enumerate(r["out_names"])
        })
    return per_core


def kernel(feature_map, concepts, w_theta, w_phi, w_g, w_o,
           gamma_sa, gamma_moca):
    in_maps = _prep_in_maps(feature_map, concepts, w_theta, w_phi, w_g, w_o,
                            gamma_sa, gamma_moca)
    # fm is batch-major contiguous; the concatenated [B*C, HW] layout is a
    # zero-copy reshape of the full input.
    fm_cat = np.ascontiguousarray(
        np.asarray(feature_map, dtype=np.float32)).reshape(B * C, HW)
    per_core = _run(in_maps, concat_override={"fm": fm_cat})
    out = np.stack([per_core[b]["out"].reshape(C, H, W)
                    for b in range(B)], axis=0)
    return out.astype(np.float32)
